# revision 3
# baseline (speedup 1.0000x reference)
"""AnchorLoss distributed Bass kernel for 8 TRN2 NeuronCores.

loss = -(2*n*sum(a^2) - 2*||colsum(a)||^2) / sqrt(dim_emb) / k^2

Strategy (data-parallel over n_classes):
  - Each core streams its [1024, 6144] f32 shard HBM->SBUF in 8 tiles of
    [128, 6144] (3 MiB per DMA; the kernel is DMA-bound at ~340 GB/s/core).
  - ScalarEngine: Square activation with accum_out -> per-partition local
    sum-of-squares, one pass per tile (~5.4 us/tile).
  - VectorEngine: casts each tile f32->bf16 (~3.3 us/tile).
  - TensorEngine: bf16 one-hot ones-matmuls accumulate the column-sum of
    all 8 tiles into one PSUM bank laid out as [12, 512]. bf16 keeps the
    PE far off the critical path (fp32 matmul is ~5x slower); the bf16
    rounding enters only through ||S||^2, which is ~1e-4 of the loss, so
    the end-to-end error contribution is ~1e-8.
  - Tiny AllReduce (24.6 KiB: colsum vector + sumsq scalar) across 8 cores.
  - Each core finishes: ||S||^2 via ACT square+accum plus a ones-matmul
    pre-scaled by -COEF, one fused DVE combine, and writes the scalar out.
"""

import math
import sys

import numpy as np

if "/opt/trn_rl_repo" not in sys.path:
    sys.path.insert(0, "/opt/trn_rl_repo")

import concourse.bacc as bacc
import concourse.bass as bass
import concourse.mybir as mybir
import concourse.tile as tile
from concourse.bass_utils import run_bass_kernel_spmd

N_CORES = 8
N_CLASSES = 8192
K_ANCH = 8
DIM_EMB = 768
D = K_ANCH * DIM_EMB           # 6144 features per class row
ROWS = N_CLASSES // N_CORES    # 1024 rows per core
P = 128
N_TILES = ROWS // P            # 8 tiles of [128, D] per core
CHUNK = 512                    # one PSUM bank of fp32 per matmul
N_CHUNKS = D // CHUNK          # 12
CC_LEN = D + 8                 # collective buffer, padded to 32B multiple
F32 = mybir.dt.float32
BF16 = mybir.dt.bfloat16
# loss = COEF * (n*sumsq - ||colsum||^2)
COEF = -2.0 / (math.sqrt(DIM_EMB) * K_ANCH * K_ANCH)


def build():
    nc = bacc.Bacc(
        "TRN2", target_bir_lowering=False, debug=False, num_devices=N_CORES
    )
    a_ext = nc.dram_tensor("anchors", [ROWS, D], F32, kind="ExternalInput")
    out_ext = nc.dram_tensor("out", [1, 1], F32, kind="ExternalOutput")

    ones_dram = nc.inline_tensor(np.ones((P, 1), dtype=np.float32), name="ones1")
    # dot-matmul weights pre-scaled by -COEF so the final combine is one op
    negc_dram = nc.inline_tensor(
        np.full((P, 1), -COEF, dtype=np.float32), name="negcoef"
    )

    with tile.TileContext(nc) as tc:
        with (
            tc.tile_pool(name="inp", bufs=4) as inp_pool,
            tc.tile_pool(name="bft", bufs=3) as bf_pool,
            tc.tile_pool(name="scr", bufs=1) as scr_pool,
            tc.tile_pool(name="small", bufs=1) as small,
            tc.tile_pool(name="psum", bufs=1, space=bass.MemorySpace.PSUM) as psum_pool,
            tc.tile_pool(name="dram", bufs=1, space=bass.MemorySpace.DRAM) as dram_pool,
        ):
            # bf16 one-hot weight matrices: oh[:, j, m] = (m == j)
            oh = small.tile([P, N_CHUNKS, N_CHUNKS], BF16)
            nc.gpsimd.memset(oh[:], 0.0)
            for j in range(N_CHUNKS):
                nc.gpsimd.memset(oh[:, j, j : j + 1], 1.0)
            ones1 = small.tile([P, 1], F32)
            nc.sync.dma_start(out=ones1[:], in_=ones_dram.ap())
            negc = small.tile([P, 1], F32)
            nc.sync.dma_start(out=negc[:], in_=negc_dram.ap())

            sq_parts = small.tile([P, N_TILES], F32)
            scratch = scr_pool.tile([P, D], F32)
            cs_psum = psum_pool.tile([N_CHUNKS, CHUNK], F32)

            a_v = a_ext.ap().rearrange("(t p) d -> t p d", p=P)
            for t in range(N_TILES):
                tl = inp_pool.tile([P, D], F32)
                nc.sync.dma_start(out=tl[:], in_=a_v[t])
                # local sum of squares along the free axis, one col per tile
                nc.scalar.activation(
                    scratch[:],
                    tl[:],
                    mybir.ActivationFunctionType.Square,
                    accum_out=sq_parts[:, t : t + 1],
                )
                # column-sum on the PE in bf16
                tb = bf_pool.tile([P, D], BF16)
                nc.vector.tensor_copy(tb[:], tl[:])
                for j in range(N_CHUNKS):
                    nc.tensor.matmul(
                        cs_psum[:],
                        oh[:, j, :],
                        tb[:, j * CHUNK : (j + 1) * CHUNK],
                        start=(t == 0 and j == 0),
                        stop=(t == N_TILES - 1 and j == N_CHUNKS - 1),
                    )

            # local sum of squares -> scalar in PSUM
            ss_loc = small.tile([P, 1], F32)
            nc.vector.reduce_sum(ss_loc[:], sq_parts[:], axis=mybir.AxisListType.X)
            ss_psum = psum_pool.tile([1, 1], F32)
            nc.tensor.matmul(ss_psum[:], ones1[:], ss_loc[:])

            # stage local partials to DRAM for the collective
            cs_sb = scr_pool.tile([N_CHUNKS, CHUNK], F32)
            nc.vector.tensor_copy(cs_sb[:], cs_psum[:])
            ss_sb = small.tile([1, 1], F32)
            nc.scalar.copy(ss_sb[:], ss_psum[:])

            cc_in = dram_pool.tile([CC_LEN], F32)
            cc_out = dram_pool.tile([CC_LEN], F32)
            nc.sync.dma_start(
                out=cc_in[0:D].rearrange("(r c) -> r c", r=N_CHUNKS), in_=cs_sb[:]
            )
            nc.sync.dma_start(
                out=cc_in[D : D + 1].rearrange("(a b) -> a b", a=1), in_=ss_sb[:]
            )

            nc.gpsimd.collective_compute(
                "AllReduce",
                mybir.AluOpType.add,
                replica_groups=[list(range(N_CORES))],
                ins=[cc_in.opt()],
                outs=[cc_out.opt()],
            )

            # global colsum S laid out [128, 48]; global sumsq scalar
            s48 = small.tile([P, D // P], F32)
            nc.sync.dma_start(
                out=s48[:], in_=cc_out[0:D].rearrange("(p f) -> p f", p=P)
            )
            gss = small.tile([1, 1], F32)
            nc.sync.dma_start(
                out=gss[:], in_=cc_out[D : D + 1].rearrange("(a b) -> a b", a=1)
            )

            # ||S||^2 via Square activation with free-axis accumulate
            sq48 = small.tile([P, D // P], F32)
            dot_p = small.tile([P, 1], F32)
            nc.scalar.activation(
                sq48[:],
                s48[:],
                mybir.ActivationFunctionType.Square,
                accum_out=dot_p[:],
            )
            # dotc = -COEF * ||S||^2
            dot_psum = psum_pool.tile([1, 1], F32)
            nc.tensor.matmul(dot_psum[:], negc[:], dot_p[:])

            # loss = (gss * COEF*n) + dotc, one fused DVE op
            res = small.tile([1, 1], F32)
            nc.vector.scalar_tensor_tensor(
                res[:],
                gss[:],
                float(COEF * N_CLASSES),
                dot_psum[:],
                op0=mybir.AluOpType.mult,
                op1=mybir.AluOpType.add,
            )
            nc.sync.dma_start(out=out_ext.ap(), in_=res[:])

    nc.compile()
    return nc


_NC_CACHE = None


def _get_nc():
    global _NC_CACHE
    if _NC_CACHE is None:
        _NC_CACHE = build()
    return _NC_CACHE


def make_in_maps(anchors: np.ndarray) -> list[dict[str, np.ndarray]]:
    a = np.ascontiguousarray(anchors, dtype=np.float32).reshape(N_CLASSES, D)
    return [
        {"anchors": np.ascontiguousarray(a[c * ROWS : (c + 1) * ROWS])}
        for c in range(N_CORES)
    ]


def kernel(anchors: np.ndarray) -> np.ndarray:
    nc = _get_nc()
    res = run_bass_kernel_spmd(
        nc, make_in_maps(anchors), core_ids=list(range(N_CORES))
    )
    out = np.asarray(res.results[0]["out"], dtype=np.float32)
    return out.reshape(())


# revision 4
# speedup vs baseline: 2.8641x; 2.8641x over previous
"""AnchorLoss distributed Bass kernel for 8 TRN2 NeuronCores.

loss = -(2*n*sum(a^2) - 2*||colsum(a)||^2) / sqrt(dim_emb) / k^2

Strategy (data-parallel over n_classes):
  - Each core streams its [1024, 6144] f32 shard HBM->SBUF in 16 tiles of
    [128, 3072] (1.5 MiB per DMA; the kernel is DMA-bound at ~350 GB/s/core).
  - ScalarEngine: Square activation with accum_out -> per-partition local
    sum-of-squares, one pass per tile.
  - VectorEngine: casts each tile f32->bf16.
  - TensorEngine: bf16 one-hot ones-matmuls accumulate the column-sum of
    all tiles into one PSUM bank laid out as [12, 512]. bf16 keeps the PE
    far off the critical path (fp32 matmul is ~5x slower); the bf16
    rounding enters only through ||S||^2, which is ~1e-4 of the loss, so
    the end-to-end error contribution is ~1e-8.
  - A dummy 32B AllReduce fires at kernel start: the first collective pays
    a ~50us ncfw barrier/init that this hides under the DMA phase, so the
    real AllReduce later runs at its ~12us floor.
  - Tiny AllReduce (24.6 KiB: colsum vector + sumsq scalar) across 8 cores.
  - Each core finishes: ||S||^2 via ACT square+accum plus a ones-matmul
    pre-scaled by -COEF, one fused DVE combine, and writes the scalar out.
"""

import math
import sys

import numpy as np

if "/opt/trn_rl_repo" not in sys.path:
    sys.path.insert(0, "/opt/trn_rl_repo")

import concourse.bacc as bacc
import concourse.bass as bass
import concourse.mybir as mybir
import concourse.tile as tile
from concourse.bass_utils import run_bass_kernel_spmd

N_CORES = 8
N_CLASSES = 8192
K_ANCH = 8
DIM_EMB = 768
D = K_ANCH * DIM_EMB           # 6144 features per class row
ROWS = N_CLASSES // N_CORES    # 1024 rows per core
P = 128
N_RTILES = ROWS // P           # 8 row tiles
N_HALVES = 2                   # column halves per row tile
HD = D // N_HALVES             # 3072
CHUNK = 512                    # one PSUM bank of fp32 per matmul
N_CHUNKS = D // CHUNK          # 12
HCHUNKS = HD // CHUNK          # 6 chunks per half
CC_LEN = D + 16                # collective buffer, padded to 32B multiple
F32 = mybir.dt.float32
BF16 = mybir.dt.bfloat16
# loss = COEF * (n*sumsq - ||colsum||^2)
COEF = -2.0 / (math.sqrt(DIM_EMB) * K_ANCH * K_ANCH)


def build():
    nc = bacc.Bacc(
        "TRN2", target_bir_lowering=False, debug=False, num_devices=N_CORES
    )
    a_ext = nc.dram_tensor("anchors", [ROWS, D], F32, kind="ExternalInput")
    out_ext = nc.dram_tensor("out", [1, 1], F32, kind="ExternalOutput")

    ones_dram = nc.inline_tensor(np.ones((P, 1), dtype=np.float32), name="ones1")
    # dot-matmul weights pre-scaled by -COEF so the final combine is one op
    negc_dram = nc.inline_tensor(
        np.full((P, 1), -COEF, dtype=np.float32), name="negcoef"
    )

    with tile.TileContext(nc) as tc:
        with (
            tc.tile_pool(name="inp", bufs=6) as inp_pool,
            tc.tile_pool(name="bft", bufs=3) as bf_pool,
            tc.tile_pool(name="scr", bufs=1) as scr_pool,
            tc.tile_pool(name="small", bufs=1) as small,
            tc.tile_pool(name="psum", bufs=1, space=bass.MemorySpace.PSUM) as psum_pool,
            tc.tile_pool(name="dram", bufs=1, space=bass.MemorySpace.DRAM) as dram_pool,
        ):
            cc_in = dram_pool.tile([CC_LEN], F32, tag="cc_in")
            cc_out = dram_pool.tile([CC_LEN], F32, tag="cc_out")
            cc_sync_in = dram_pool.tile([8], F32, tag="cc_sync_in")
            cc_sync_out = dram_pool.tile([8], F32, tag="cc_sync_out")

            # Warm-up collective: absorbs the ncfw first-collective barrier
            # (~50us) concurrently with the DMA/compute phase below. Its
            # result is summed launch-skew garbage; it lands in cc_in's pad
            # lanes, which nothing reads.
            nc.gpsimd.collective_compute(
                "AllReduce",
                mybir.AluOpType.add,
                replica_groups=[list(range(N_CORES))],
                ins=[cc_sync_in.opt()],
                outs=[cc_sync_out.opt()],
            )
            nc.gpsimd.dma_start(
                out=cc_in[D + 8 : D + 16].rearrange("(a b) -> a b", a=1),
                in_=cc_sync_out[:].rearrange("(a b) -> a b", a=1),
            )

            # bf16 one-hot weight matrices: oh[:, j, m] = (m == j)
            oh = small.tile([P, N_CHUNKS, N_CHUNKS], BF16)
            nc.gpsimd.memset(oh[:], 0.0)
            for j in range(N_CHUNKS):
                nc.gpsimd.memset(oh[:, j, j : j + 1], 1.0)

            sq_parts = small.tile([P, N_RTILES * N_HALVES], F32)
            scratch = scr_pool.tile([P, HD], F32)
            cs_psum = psum_pool.tile([N_CHUNKS, CHUNK], F32)

            a_v = a_ext.ap().rearrange("(t p) d -> t p d", p=P)
            n_total = N_RTILES * N_HALVES
            for i in range(n_total):
                t, h = divmod(i, N_HALVES)
                tl = inp_pool.tile([P, HD], F32)
                nc.sync.dma_start(out=tl[:], in_=a_v[t][:, h * HD : (h + 1) * HD])
                # local sum of squares along the free axis, one col per tile
                nc.scalar.activation(
                    scratch[:],
                    tl[:],
                    mybir.ActivationFunctionType.Square,
                    accum_out=sq_parts[:, i : i + 1],
                )
                # column-sum on the PE in bf16
                tb = bf_pool.tile([P, HD], BF16)
                nc.vector.tensor_copy(tb[:], tl[:])
                for j in range(HCHUNKS):
                    jj = h * HCHUNKS + j
                    nc.tensor.matmul(
                        cs_psum[:],
                        oh[:, jj, :],
                        tb[:, j * CHUNK : (j + 1) * CHUNK],
                        start=(i == 0 and j == 0),
                        stop=(i == n_total - 1 and j == HCHUNKS - 1),
                    )

            # constants for the tail (loaded late: not needed until here)
            ones1 = small.tile([P, 1], F32)
            nc.sync.dma_start(out=ones1[:], in_=ones_dram.ap())
            negc = small.tile([P, 1], F32)
            nc.sync.dma_start(out=negc[:], in_=negc_dram.ap())

            # local sum of squares -> scalar in PSUM
            ss_loc = small.tile([P, 1], F32)
            nc.vector.reduce_sum(ss_loc[:], sq_parts[:], axis=mybir.AxisListType.X)
            ss_psum = psum_pool.tile([1, 1], F32)
            nc.tensor.matmul(ss_psum[:], ones1[:], ss_loc[:])

            # stage local partials to DRAM for the collective
            cs_sb = scr_pool.tile([N_CHUNKS, CHUNK], F32)
            nc.vector.tensor_copy(cs_sb[:], cs_psum[:])
            ss_sb = small.tile([1, 1], F32)
            nc.scalar.copy(ss_sb[:], ss_psum[:])

            nc.sync.dma_start(
                out=cc_in[0:D].rearrange("(r c) -> r c", r=N_CHUNKS), in_=cs_sb[:]
            )
            nc.sync.dma_start(
                out=cc_in[D : D + 1].rearrange("(a b) -> a b", a=1), in_=ss_sb[:]
            )

            nc.gpsimd.collective_compute(
                "AllReduce",
                mybir.AluOpType.add,
                replica_groups=[list(range(N_CORES))],
                ins=[cc_in.opt()],
                outs=[cc_out.opt()],
            )

            # global colsum S laid out [128, 48]; global sumsq scalar
            s48 = small.tile([P, D // P], F32)
            nc.sync.dma_start(
                out=s48[:], in_=cc_out[0:D].rearrange("(p f) -> p f", p=P)
            )
            gss = small.tile([1, 1], F32)
            nc.sync.dma_start(
                out=gss[:], in_=cc_out[D : D + 1].rearrange("(a b) -> a b", a=1)
            )

            # ||S||^2 via Square activation with free-axis accumulate
            sq48 = small.tile([P, D // P], F32)
            dot_p = small.tile([P, 1], F32)
            nc.scalar.activation(
                sq48[:],
                s48[:],
                mybir.ActivationFunctionType.Square,
                accum_out=dot_p[:],
            )
            # dotc = -COEF * ||S||^2
            dot_psum = psum_pool.tile([1, 1], F32)
            nc.tensor.matmul(dot_psum[:], negc[:], dot_p[:])

            # loss = (gss * COEF*n) + dotc, one fused DVE op
            res = small.tile([1, 1], F32)
            nc.vector.scalar_tensor_tensor(
                res[:],
                gss[:],
                float(COEF * N_CLASSES),
                dot_psum[:],
                op0=mybir.AluOpType.mult,
                op1=mybir.AluOpType.add,
            )
            nc.sync.dma_start(out=out_ext.ap(), in_=res[:])

    nc.compile()
    return nc


_NC_CACHE = None


def _get_nc():
    global _NC_CACHE
    if _NC_CACHE is None:
        _NC_CACHE = build()
    return _NC_CACHE


def make_in_maps(anchors: np.ndarray) -> list[dict[str, np.ndarray]]:
    a = np.ascontiguousarray(anchors, dtype=np.float32).reshape(N_CLASSES, D)
    return [
        {"anchors": np.ascontiguousarray(a[c * ROWS : (c + 1) * ROWS])}
        for c in range(N_CORES)
    ]


def kernel(anchors: np.ndarray) -> np.ndarray:
    nc = _get_nc()
    res = run_bass_kernel_spmd(
        nc, make_in_maps(anchors), core_ids=list(range(N_CORES))
    )
    out = np.asarray(res.results[0]["out"], dtype=np.float32)
    return out.reshape(())
